# revision 6
# baseline (speedup 1.0000x reference)
"""Trainium2 Bass kernel for nn_EnhancedSpatialTransformerEncoder.

Strategy: data-parallel over batch (32 items -> 4 per NeuronCore, 8 cores).
Each core runs the full 4-layer encoder on its 4 items with all weights
replicated in SBUF.

Layout: activations are kept feature-major ([D, T] with features on SBUF
partitions) so every projection matmul contracts on the partition dim and
the stored [d_in, d_out] weights serve directly as lhsT.

Attention: scores are computed transposed ([Tk, Tq]) per head with 4-way
row-packed K=32 matmuls.  Softmax skips max-subtraction (safe: the additive
spatial bias dominates and exp stays inside fp32 range) and uses
exp(s + b) = exp(s) * exp(b) with exp(bias^T) precomputed on the host.
Row sums ride a col-packed ones-matmul whose output rows align exactly with
the col-packed unnormalized O^T tile, so normalization is two full-width
vector ops.

LayerNorm (over the partition dim): moment matmuls against a constant
1/256 matrix produce broadcast mean/var tiles directly in PSUM;
rsqrt(var+eps) = exp(-0.5*ln(var+eps)) on the scalar engine, which keeps a
single activation-table set (exp+ln) loaded for the whole kernel.

Matmuls run in bf16 with fp32 PSUM accumulation.
"""

import numpy as np
import ml_dtypes

import bass_rust
import concourse.bass as bass
import concourse.tile as tile
import concourse.mybir as mybir
from concourse.bass_utils import run_bass_kernel_spmd
from concourse.vector_clock import ScopedClock

# ---------------------------------------------------------------- dims
L = 4        # layers
B = 32       # batch
N = 512      # seq len
D = 256      # d_model
H = 8        # heads
HD = 32      # head dim
F = 1024     # ffn dim
EPS = 1e-5
P = 128
NCORES = 8
IB = B // NCORES          # items per core
DK = D // P               # 2  feature K-tiles
FK = F // P               # 8  ffn K-tiles
TT = N // P               # 4  token tiles

BF16 = mybir.dt.bfloat16
F32 = mybir.dt.float32
AF = mybir.ActivationFunctionType
ALU = mybir.AluOpType
bf16 = ml_dtypes.bfloat16


class _TileContextSplitDrain(tile.TileContext):
    """TileContext whose final drain splits its sem waits across sync NOPs.

    The installed walrus rejects >1 sync wait command on a CTRL-format
    Drain ("Too many sync wait commands"), so put each wait on its own NOP
    ahead of the barrier instead.
    """

    def _drain_and_barrier(self, tick_clock, wait_clock):
        nc = self.nc
        drain_inst = nc.sync.drain()
        wait_clock.add_sem_waits(
            drain_inst.ins, ScopedClock({None: tick_clock.global_clock})
        )
        si = drain_inst.ins.sync_info
        waits = list(si.on_wait) if si is not None and si.on_wait else []
        if len(waits) > 1:
            si.on_wait = [waits[0]]
            for w in waits[1:]:
                nop = nc.sync.nop(nofuse=True)
                nop.ins.sync_info = mybir.SyncInfo(on_wait=[w], on_update=[])

        nc.all_engine_barrier()
        assert self.sems is not None
        popped = nc._tile_sem_poison_stack.pop()
        assert popped is self._sem_poison
        nc.clear_and_free_semaphores(list(self.sems.allocated().values()))
        nc.all_engine_barrier()


def _split_multiwaits(nc):
    """Hoist extra sem waits onto same-engine NOPs.

    The installed walrus rejects more than one sync-wait command per
    instruction ("Too many sync wait commands"), while Tile's sem
    assignment freely attaches several.  Rewrite every block so each
    instruction carries at most one wait; extra waits go on NOPs placed
    immediately before it in the same engine's stream.
    """
    skip = (mybir.InstAllEngineBarrier,)
    ctr = [0]
    for fn in nc.m.functions:
        for bb in fn.blocks:
            new = []
            for inst in bb.instructions:
                si = inst.sync_info
                if (si is not None and si.on_wait and len(si.on_wait) > 1
                        and not isinstance(inst, skip)):
                    waits = list(si.on_wait)
                    for w in waits[:-1]:
                        ctr[0] += 1
                        nop = bass_rust.InstNoOp(
                            name=f"I-waitsplit-{ctr[0]}", ins=[], outs=[])
                        nop.engine = inst.engine
                        nop.sync_info = mybir.SyncInfo(on_wait=[w], on_update=[])
                        new.append(nop)
                    si.on_wait = [waits[-1]]
                new.append(inst)
            bb.instructions = new


# Column layout of the packed per-layer [128, 26] fp32 bias/scale sheet.
# Each named vector of length 256 occupies 2 columns (feature tiles).
_VEC_NAMES = ["bq", "bk", "bo", "b2", "g1", "be1", "g2", "be2"]  # 16 cols
_B1_COL = 16  # b1 occupies cols 16..23 (8 F-tiles)
_VCOLS = 24


def _build_module():
    nc = bass.Bass()

    srcT = nc.dram_tensor("srcT", [IB, DK, P, N], BF16, kind="ExternalInput")
    expB = nc.dram_tensor("expB", [P, TT, N], BF16, kind="ExternalInput")
    wq = nc.dram_tensor("wq", [L, DK, P, D], BF16, kind="ExternalInput")
    wk = nc.dram_tensor("wk", [L, DK, P, D], BF16, kind="ExternalInput")
    wv = nc.dram_tensor("wv", [L, DK, P, D], BF16, kind="ExternalInput")
    wo = nc.dram_tensor("wo", [L, DK, P, D], BF16, kind="ExternalInput")
    w1 = nc.dram_tensor("w1", [L, DK, P, F], BF16, kind="ExternalInput")
    w2 = nc.dram_tensor("w2", [L, FK, P, D], BF16, kind="ExternalInput")
    bvr = nc.dram_tensor("bvr", [L, P, D], BF16, kind="ExternalInput")
    vecs = nc.dram_tensor("vecs", [L, P, _VCOLS], F32, kind="ExternalInput")
    muw = nc.dram_tensor("muw", [P, P], BF16, kind="ExternalInput")
    ones32 = nc.dram_tensor("ones32", [P, HD], BF16, kind="ExternalInput")
    ident = nc.dram_tensor("ident", [P, P], F32, kind="ExternalInput")
    epsb = nc.dram_tensor("epsb", [P, 1], F32, kind="ExternalInput")
    out = nc.dram_tensor("out", [IB, N, D], F32, kind="ExternalOutput")

    with _TileContextSplitDrain(nc) as tc:
        with (
            tc.tile_pool(name="const", bufs=1) as cpool,
            tc.tile_pool(name="work", bufs=2) as wk_pool,
            tc.tile_pool(name="ps", bufs=2, space="PSUM") as ps_pool,
        ):
            _emit(nc, tc, cpool, wk_pool, ps_pool, dict(
                srcT=srcT, expB=expB, wq=wq, wk=wk, wv=wv, wo=wo, w1=w1,
                w2=w2, bvr=bvr, vecs=vecs, muw=muw, ones32=ones32,
                ident=ident, epsb=epsb, out=out,
            ))
    _split_multiwaits(nc)
    return nc


def _emit(nc, tc, cpool, wk_pool, ps_pool, io):
    dma = nc.sync.dma_start

    def ctile(shape, dtype, tag):
        return cpool.tile(shape, dtype, tag=tag, name=tag)

    def wtile(shape, dtype, tag, bufs):
        return wk_pool.tile(shape, dtype, tag=tag, bufs=bufs, name=tag)

    def ptile(shape, tag, bufs):
        return ps_pool.tile(shape, F32, tag=tag, bufs=bufs, name=tag)

    # ---------------- constants ----------------
    expB_sb = ctile([P, TT, N], BF16, "expB")
    dma(out=expB_sb, in_=io["expB"][:, :, :])
    muw_sb = ctile([P, P], BF16, "muw")
    dma(out=muw_sb, in_=io["muw"][:, :])
    ones32_sb = ctile([P, HD], BF16, "ones32")
    dma(out=ones32_sb, in_=io["ones32"][:, :])
    ident_sb = ctile([P, P], F32, "ident")
    dma(out=ident_sb, in_=io["ident"][:, :])
    eps_sb = ctile([P, 1], F32, "epsb")
    dma(out=eps_sb, in_=io["epsb"][:, :])

    W = {}
    for l in range(L):
        for name, kt_n, width in (("wq", DK, D), ("wk", DK, D), ("wv", DK, D),
                                  ("wo", DK, D), ("w1", DK, F), ("w2", FK, D)):
            tiles = []
            for kt in range(kt_n):
                t = ctile([P, width], BF16, f"{name}{l}_{kt}")
                dma(out=t, in_=io[name][l, kt, :, :])
                tiles.append(t)
            W[(name, l)] = tiles
        t = ctile([P, D], BF16, f"bvr{l}")
        dma(out=t, in_=io["bvr"][l, :, :])
        W[("bvr", l)] = t
        t = ctile([P, _VCOLS], F32, f"vecs{l}")
        dma(out=t, in_=io["vecs"][l, :, :])
        W[("vecs", l)] = t

    def vec(l, name, kt):
        col = (_B1_COL + kt) if name == "b1" else (2 * _VEC_NAMES.index(name) + kt)
        return W[("vecs", l)][:, col:col + 1]

    # ---------------- per-item pipeline ----------------
    def layernorm(l, r_tiles, gname, bname, out_dtype, out_tag, out_bufs):
        mu = ptile([P, N], "acc", 4)
        for kt in range(DK):
            nc.tensor.matmul(mu, lhsT=muw_sb, rhs=r_tiles[kt],
                             start=(kt == 0), stop=(kt == DK - 1))
        cs, c2s = [], []
        for kt in range(DK):
            c = wtile([P, N], BF16, "c", 5)
            nc.vector.tensor_sub(c, r_tiles[kt], mu)
            c2 = wtile([P, N], BF16, "c2", 4)
            nc.vector.tensor_mul(c2, c, c)
            cs.append(c)
            c2s.append(c2)
        var = ptile([P, N], "acc", 4)
        for kt in range(DK):
            nc.tensor.matmul(var, lhsT=muw_sb, rhs=c2s[kt],
                             start=(kt == 0), stop=(kt == DK - 1))
        lnv = wtile([P, N], F32, "lnv", 2)
        nc.scalar.activation(lnv, var, AF.Ln, bias=eps_sb)
        rho = wtile([P, N], BF16, "rho", 2)
        nc.scalar.activation(rho, lnv, AF.Exp, scale=-0.5)
        outs = []
        for kt in range(DK):
            y = wtile([P, N], out_dtype, out_tag, out_bufs)
            nc.vector.scalar_tensor_tensor(
                out=y, in0=cs[kt], scalar=vec(l, gname, kt), in1=rho,
                op0=ALU.mult, op1=ALU.mult)
            nc.vector.tensor_scalar(out=y, in0=y, scalar1=vec(l, bname, kt),
                                    scalar2=None, op0=ALU.add)
            outs.append(y)
        return outs

    for item in range(IB):
        x = []
        for kt in range(DK):
            t = wtile([P, N], BF16, "x", 5)
            dma(out=t, in_=io["srcT"][item, kt, :, :])
            x.append(t)

        for l in range(L):
            # ---------- q/k projections (feature-major) ----------
            qk = {}
            for name in ("wq", "wk"):
                tiles = []
                for mt in range(DK):
                    ps = ptile([P, N], "acc", 4)
                    for kt in range(DK):
                        nc.tensor.matmul(
                            ps, lhsT=W[(name, l)][kt][:, P * mt:P * (mt + 1)],
                            rhs=x[kt], start=(kt == 0), stop=(kt == DK - 1))
                    t = wtile([P, N], BF16, "qk", 6)
                    bn = "bq" if name == "wq" else "bk"
                    nc.vector.tensor_scalar(out=t, in0=ps, scalar1=vec(l, bn, mt),
                                            scalar2=None, op0=ALU.add)
                    tiles.append(t)
                qk[name] = tiles
            qT, kT = qk["wq"], qk["wk"]

            # ---------- v projection (token-major) ----------
            v = []
            for tt in range(TT):
                ps = ptile([P, D], "acc", 4)
                for kt in range(DK):
                    nc.tensor.matmul(
                        ps, lhsT=x[kt][:, P * tt:P * (tt + 1)],
                        rhs=W[("wv", l)][kt], start=(kt == 0), stop=(kt == DK - 1))
                t = wtile([P, D], BF16, "v", 6)
                nc.vector.tensor_add(t, ps, W[("bvr", l)])
                v.append(t)

            # ---------- attention ----------
            ohat_sb = []
            for g in range(2):
                heads = [4 * g + i for i in range(4)]
                attn = {h: wtile([P, TT, N], BF16, "attn", 6) for h in heads}
                for j in range(TT):
                    for hh, h in enumerate(heads):
                        sps = ptile([P, N], "scores", 4)
                        nc.tensor.matmul(
                            sps,
                            lhsT=kT[g][HD * hh:HD * (hh + 1), P * j:P * (j + 1)],
                            rhs=qT[g][HD * hh:HD * (hh + 1), :],
                            start=True, stop=True,
                            tile_position=(HD * hh, 0))
                        nc.scalar.activation(attn[h][:, j, :], sps, AF.Exp)
                for h in heads:
                    nc.vector.tensor_mul(attn[h], attn[h], expB_sb)

                oh_ps = ptile([P, N], "acc", 4)
                sm_ps = ptile([P, N], "acc", 4)
                for j in range(TT):
                    for hh, h in enumerate(heads):
                        nc.tensor.matmul(
                            oh_ps[HD * hh:HD * (hh + 1), :],
                            lhsT=v[j][:, HD * h:HD * (h + 1)],
                            rhs=attn[h][:, j, :],
                            start=(j == 0), stop=(j == TT - 1),
                            tile_position=(0, HD * hh),
                            skip_group_check=True)
                for j in range(TT):
                    for hh, h in enumerate(heads):
                        nc.tensor.matmul(
                            sm_ps[HD * hh:HD * (hh + 1), :],
                            lhsT=ones32_sb,
                            rhs=attn[h][:, j, :],
                            start=(j == 0), stop=(j == TT - 1),
                            tile_position=(0, HD * hh),
                            skip_group_check=True)
                inv = wtile([P, N], F32, "inv", 2)
                nc.vector.reciprocal(inv, sm_ps)
                oh = wtile([P, N], BF16, "oh", 3)
                nc.vector.tensor_mul(oh, oh_ps, inv)
                ohat_sb.append(oh)

            # ---------- output projection + residual ----------
            r1 = []
            for mt in range(DK):
                ps = ptile([P, N], "acc", 4)
                for kt in range(DK):
                    nc.tensor.matmul(
                        ps, lhsT=W[("wo", l)][kt][:, P * mt:P * (mt + 1)],
                        rhs=ohat_sb[kt], start=(kt == 0), stop=(kt == DK - 1))
                r = wtile([P, N], BF16, "r", 5)
                nc.vector.scalar_tensor_tensor(
                    out=r, in0=ps, scalar=vec(l, "bo", mt), in1=x[mt],
                    op0=ALU.add, op1=ALU.add)
                r1.append(r)

            y1 = layernorm(l, r1, "g1", "be1", BF16, "y1", 5)

            # ---------- ffn ----------
            h1 = []
            for mt in range(FK):
                ps = ptile([P, N], "acc", 4)
                for kt in range(DK):
                    nc.tensor.matmul(
                        ps, lhsT=W[("w1", l)][kt][:, P * mt:P * (mt + 1)],
                        rhs=y1[kt], start=(kt == 0), stop=(kt == DK - 1))
                t = wtile([P, N], BF16, "h1", 10)
                nc.vector.tensor_scalar(out=t, in0=ps, scalar1=vec(l, "b1", mt),
                                        scalar2=0.0, op0=ALU.add, op1=ALU.max)
                h1.append(t)
            r2 = []
            for mt in range(DK):
                ps = ptile([P, N], "acc", 4)
                for kt in range(FK):
                    nc.tensor.matmul(
                        ps, lhsT=W[("w2", l)][kt][:, P * mt:P * (mt + 1)],
                        rhs=h1[kt], start=(kt == 0), stop=(kt == FK - 1))
                r = wtile([P, N], BF16, "r", 5)
                nc.vector.scalar_tensor_tensor(
                    out=r, in0=ps, scalar=vec(l, "b2", mt), in1=y1[mt],
                    op0=ALU.add, op1=ALU.add)
                r2.append(r)

            if l < L - 1:
                x = layernorm(l, r2, "g2", "be2", BF16, "x", 5)
            else:
                yf = layernorm(l, r2, "g2", "be2", F32, "yf", 3)

        # ---------- transpose back to [T, D] and store ----------
        for tt in range(TT):
            yt = wtile([P, D], F32, "yt", 4)
            for a in range(DK):
                tp = ptile([P, P], "acc", 4)
                nc.tensor.transpose(tp, yf[a][:, P * tt:P * (tt + 1)], ident_sb)
                nc.vector.tensor_copy(yt[:, P * a:P * (a + 1)], tp)
            dma(out=io["out"][item, P * tt:P * (tt + 1), :], in_=yt)


_NC_CACHE = None


def _get_module():
    global _NC_CACHE
    if _NC_CACHE is None:
        _NC_CACHE = _build_module()
    return _NC_CACHE


def kernel(src, e1, e2, Wq, bq, Wk, bk, Wv, bv, Wo, bo,
           W1, b1, W2, b2, g1, be1, g2, be2):
    src = np.asarray(src, np.float32)
    f32 = lambda a: np.asarray(a, np.float32)
    scale = 1.0 / np.sqrt(np.float32(HD))

    # exp of the transposed shared spatial bias, [Tk, Tq] -> [128, TT, N]
    biasT = (f32(e1) @ f32(e2).T).T.astype(np.float32)
    expB = np.exp(biasT).reshape(TT, P, N).transpose(1, 0, 2).astype(bf16)

    def pack_w(w, kt_n):  # [L, d_in, d_out] -> [L, kt, 128, d_out]
        w = f32(w)
        return np.ascontiguousarray(
            w.reshape(L, kt_n, P, w.shape[2])).astype(bf16)

    wq_h = pack_w(f32(Wq) * scale, DK)
    wk_h = pack_w(Wk, DK)
    wv_h = pack_w(Wv, DK)
    wo_h = pack_w(Wo, DK)
    w1_h = pack_w(W1, DK)
    w2_h = pack_w(W2, FK)

    bvr = np.broadcast_to(f32(bv)[:, None, :], (L, P, D)).astype(bf16)
    bvr = np.ascontiguousarray(bvr)

    # packed per-layer bias/scale sheet [L, 128, 24]
    vecs = np.zeros((L, P, _VCOLS), np.float32)
    named = {"bq": f32(bq) * scale, "bk": f32(bk), "bo": f32(bo),
             "b2": f32(b2), "g1": f32(g1), "be1": f32(be1),
             "g2": f32(g2), "be2": f32(be2)}
    for i, name in enumerate(_VEC_NAMES):
        a = named[name].reshape(L, DK, P)
        for kt in range(DK):
            vecs[:, :, 2 * i + kt] = a[:, kt, :]
    b1r = f32(b1).reshape(L, FK, P)
    for kt in range(FK):
        vecs[:, :, _B1_COL + kt] = b1r[:, kt, :]

    muw = np.full((P, P), 1.0 / D, np.float32).astype(bf16)
    ones32 = np.ones((P, HD), np.float32).astype(bf16)
    ident = np.eye(P, dtype=np.float32)
    epsb = np.full((P, 1), EPS, np.float32)

    # feature-major src per item: [B, N, D] -> [B, DK, 128, N]
    srcT = np.ascontiguousarray(
        src.transpose(0, 2, 1).reshape(B, DK, P, N)).astype(bf16)

    shared = dict(expB=expB, wq=wq_h, wk=wk_h, wv=wv_h, wo=wo_h, w1=w1_h,
                  w2=w2_h, bvr=bvr, vecs=vecs, muw=muw, ones32=ones32,
                  ident=ident, epsb=epsb)
    in_maps = [
        dict(shared, srcT=np.ascontiguousarray(srcT[c * IB:(c + 1) * IB]))
        for c in range(NCORES)
    ]

    nc = _get_module()
    res = run_bass_kernel_spmd(nc, in_maps, core_ids=list(range(NCORES)))
    return np.concatenate([r["out"] for r in res.results], axis=0)
